# revision 1
# baseline (speedup 1.0000x reference)
"""Trainium2 Bass kernel for KeypointSelector:
conv3x3(384->128, pad 1) + bias + ReLU -> conv1x1(128->1) + bias + sigmoid.

Input  dino_features: (32, 64, 64, 384) f32
Output (32, 64, 64, 1) f32

Strategy: pure data parallel over batch, 4 images per core on 8 cores.
All matmul work runs in fp8e4m3 with MatmulPerfMode.DoubleRow (0.5
PE cycles per output column, 4x the bf16 column rate):

- Conv3x3: the 27 contraction k-tiles (3 cin chunks x 9 taps) plus a
  bias k-tile pack exactly into 14 DoubleRow pairs. Both slices of every
  pair live at a constant column delta inside one flat SBUF image tile
  [128, 3*4356 + ones], so a pair is a single 3-dim access pattern; all
  deltas exceed the 512-column moving tile (no overlapping reads). The
  conv bias b1 is folded into pair 13, whose second slice reads an
  all-ones region with weights 4096*b1/128.
- ReLU + fp8 cast runs on the otherwise-idle DVE as one
  scalar_tensor_tensor: h = max(psum * 1/4096, 0).
- Conv1x1 is also DoubleRow: dual-fp8 LDWEIGHTS requires >=16 stationary
  columns and dst partition 0, so w2*16 occupies (slot 0, col 0) of a
  [128, 2, 16] zero block and the rhs duplicates h via a stride-0 k-tile
  dim. Sigmoid (ACT) applies the 1/16 descale and partition-shifts each
  tile's row so one strided DMA per group writes the output.

fp8 scaling: x*16 and w1*256 host-side (unscaled w1 ~1e-2 would land in
the fp8 subnormal range); measured end-to-end rel err ~8e-3 vs the f32
reference (gate 2e-2).

Tiling: the conv moving AP is 4-dim [partition, k-pair, row, col], so
only the 4096 valid output pixels stream through the PE (the 2 pad
columns between rows are skipped): 8 tiles of 8 rows x 64 cols = 512
columns, exactly one PSUM bank, no partial tiles, and the output lands
densely packed.

Pipelining: tiles are processed in weight-reuse groups of 2 (pair loop
outermost, so each LDWEIGHTS serves the group); a group's relu/1x1/
sigmoid chain is emitted after the next group's conv matmuls so the PE
never queues behind other engines. Steady state sims at 51.4us/rep with
the PE 95% busy, which is the cost-model matmul floor (51.2us). PSUM
budget is exactly 8 banks: 2x2 conv accumulators + 2x2 1x1 banks. Cold
start hides the p-state ramp behind priming matmuls on a zeros tile
while the first image's DMAs land.
"""

import ml_dtypes
import numpy as np

import concourse.tile as tile
from concourse import bacc, mybir
from concourse.ap import AP
from concourse.bass_utils import run_bass_kernel_spmd

E4M3 = ml_dtypes.float8_e4m3  # == mybir.dt.np(float8e4)

# Geometry
B, H, W, CIN, CHID = 32, 64, 64, 384, 128
NCORES = 8
BLOC = B // NCORES  # 4 images per core
HP, WP = H + 2, W + 2  # 66x66 padded grid
NPIX = HP * WP  # 4356 padded pixels per image
TS = 512  # matmul free-dim tile (one PSUM bank of fp32)
START = WP + 1  # padded idx of first valid output pixel (1,1) = 67
ROWS_PT = 8  # output rows per tile; 8 rows x 64 cols = 512 = exactly TS
NT = H // ROWS_PT  # 8 tiles per image, all full
TILE_N = [TS] * NT
ROWPITCH = ROWS_PT * WP  # padded-idx stride between tiles (528)
NCHUNK = CIN // 128  # 3 cin chunks
GROUPS = [[0, 1], [2, 3], [4, 5], [6, 7]]  # weight-reuse tile groups
OUTW = NT * TS  # densely packed flat output row (4096 = 64*64)

# fp8 scaling
SX, SW = 16.0, 256.0
DESCALE = 1.0 / (SX * SW)
SW2 = 16.0  # 1x1 weight pre-scale for fp8 range
W2DESCALE = 1.0 / SW2

# k-tile pairing for DoubleRow: 27 (chunk, tap) k-tiles + 1 bias slot -> 14
# pairs. Tap t=(dy+1)*3+(dx+1) reads at padded-col offset OFF[t]; chunk c
# lives at flat offset c*NPIX. Pairs are chosen cross-chunk so every rhs
# delta is >= NPIX-134 > 512 (no overlapping access patterns).
#
# The image tile carries an all-ones region at flat offset 3*NPIX, aligned
# so that ones-col = 3*NPIX + (s0 - START) + j for output column j. Pair 13
# pairs the leftover k-tile (chunk2, tap8) with the ones region whose
# weights hold 4096*b1/128 per row -- the conv bias folded into the matmul.
OFF = [-WP - 1, -WP, -WP + 1, -1, 0, 1, WP - 1, WP, WP + 1]
ONES_W = 528  # per-tile delta aims all tiles at one 526-col ones window
XFW = NCHUNK * NPIX + ONES_W  # flat image tile width
PAIRS = (
    [(0, t, 1, t) for t in range(5)]
    + [(0, 5 + j, 2, j) for j in range(4)]
    + [(1, 5 + j, 2, 4 + j) for j in range(4)]
    + [(2, 8, None, None)]  # slot B reads the ones region (bias fold)
)
NPAIR = len(PAIRS)  # 14
BASEA, DELTA = [], []
for cA, tA, cB, tB in PAIRS:
    if cB is None:
        BASEA.append(cA * NPIX + OFF[tA])
        DELTA.append(None)  # per-tile: 3*NPIX - (BASEA[13] + s0)
    else:
        BASEA.append(cA * NPIX + OFF[tA])
        DELTA.append((cB - cA) * NPIX + OFF[tB] - OFF[tA])
assert all(d is None or d > TS for d in DELTA)

_CACHED = {}


def _build_bass(reps=1):
    nc = bacc.Bacc("TRN2", target_bir_lowering=False)

    f32 = mybir.dt.float32
    bf16 = mybir.dt.bfloat16
    fp8 = mybir.dt.float8e4
    DR = mybir.MatmulPerfMode.DoubleRow

    x = nc.dram_tensor("x", [BLOC, NCHUNK, 128, NPIX], fp8, kind="ExternalInput")
    w1 = nc.dram_tensor("w1", [128, NPAIR, 2, CHID], fp8, kind="ExternalInput")
    ones = nc.dram_tensor("ones", [128, ONES_W], fp8, kind="ExternalInput")
    # 1x1 weights for the DoubleRow trick: dual-fp8 LDWEIGHTS requires >=16
    # stationary columns, so w2*16 sits in (slot 0, col 0) of a [128, 2, 16]
    # zero block. The rhs duplicates h via a stride-0 k-tile dim (slot-1
    # weights are zero), giving 0.5 cycles/column without repacking h.
    w2 = nc.dram_tensor("w2", [128, 2, 16], fp8, kind="ExternalInput")
    b2 = nc.dram_tensor("b2", [97, 1], f32, kind="ExternalInput")
    y = nc.dram_tensor("y", [BLOC, OUTW], f32, kind="ExternalOutput")

    with tile.TileContext(nc) as tc:
        with (
            tc.tile_pool(name="consts", bufs=1) as consts,
            tc.tile_pool(name="xin", bufs=2) as xin,
            tc.tile_pool(name="hbuf", bufs=4) as hbuf,
            tc.tile_pool(name="obuf", bufs=2) as obuf,
            tc.tile_pool(name="ps1", bufs=2, space="PSUM") as ps1,
            tc.tile_pool(name="ps2", bufs=2, space="PSUM") as ps2,
        ):
            # Cold-start critical path: only w1 (needed by the first
            # Ldweights) is loaded now, on the ACT queue -- the DMA pipe
            # round-robins between queues, so it interleaves with the SP
            # queue's image chunks instead of delaying them. The small
            # consts (first needed ~9us in) are deferred until after image
            # 0's DMAs are enqueued.
            w1_s = consts.tile([128, NPAIR, 2, CHID], fp8)
            nc.scalar.dma_start(out=w1_s[:], in_=w1[:])
            # Zeros tile: second operand of the DVE relu (max against 0),
            # and fodder for the p-state priming matmuls below.
            z_s = consts.tile([CHID, TS], bf16)
            nc.scalar.memzero(z_s[:])
            w2_s = consts.tile([128, 2, 16], fp8)
            b2_s = consts.tile([97, 1], f32)  # b2 replicated host-side

            def load_small_consts():
                nc.scalar.dma_start(out=w2_s, in_=w2[:])
                nc.scalar.dma_start(out=b2_s, in_=b2[:])

            # Prime the PE p-state ramp (full clock needs ~3us of continuous
            # busy) with throwaway matmuls on the zeros tile while the first
            # image's DMAs are in flight.
            prime = ps2.tile([CHID, TS], f32, name="prime", tag="p2_0")
            for _ in range(6):
                nc.tensor.matmul(out=prime[:, :TS], lhsT=z_s[:, 0:CHID],
                                 rhs=z_s[:, 0:TS], start=True, stop=True)

            # One-group software pipeline: group g's ReLUs/1x1s are emitted
            # after group g+1's conv matmuls so the PE never queues behind
            # the ACT engine. `pend` carries [(h_s, n, tile_idx)] and img.
            #
            # Each 1x1 runs as a DoubleRow fp8 matmul at 0.5 cycles/column:
            # dual-fp8 LDWEIGHTS needs >=16 stationary cols and dst
            # partition 0, so each 1x1 gets its own PSUM bank writing rows
            # 0..15 (row 0 real, rest zero filler). The sigmoid then shifts
            # the result to out_g row 32g so one strided DMA per group
            # writes y.
            pend = None

            def flush(pend):
                hh, spans, img = pend
                hstr = list(hh.ap)[0][0]
                p2s = []
                for g, n, t in spans:
                    p2 = ps2.tile([16, TS], f32, name=f"p2_{g}",
                                  tag=f"p2_{g}")
                    rhs = AP(hh.tensor, hh.offset + g * TS,
                             [[hstr, 128], [0, 2], [1, n]])
                    nc.tensor.matmul(
                        out=p2[0:16, :n], lhsT=w2_s[:],
                        rhs=rhs, start=True, stop=True, perf_mode=DR)
                    p2s.append(p2)
                out_g = obuf.tile([97, TS], f32, name="og", tag="og")
                ostr = list(out_g.ap)[0][0]
                for (g, n, t), p2 in zip(spans, p2s):
                    nc.scalar.activation(
                        out=out_g[32 * g:32 * g + 1, :n],
                        in_=p2[0:1, :n],
                        func=mybir.ActivationFunctionType.Sigmoid,
                        bias=b2_s[0:1], scale=W2DESCALE,
                    )
                full = [(g, t) for g, n, t in spans if n == TS]
                part = [(g, n, t) for g, n, t in spans if n != TS]
                if full:
                    # DMA APs may stride partitions (engine APs may not):
                    # one strided DMA writes the group's full tiles.
                    m = len(full)
                    t0 = full[0][1]
                    nc.sync.dma_start(
                        out=y[img, t0 * TS:(t0 + m) * TS],
                        in_=AP(out_g.tensor, out_g.offset,
                               [[32 * ostr, m], [1, TS]]))
                for g, n, t in part:
                    nc.sync.dma_start(
                        out=y[img, t * TS:t * TS + n],
                        in_=out_g[32 * g:32 * g + 1, :n])

            iters = [ii for _ in range(reps) for ii in range(BLOC)]
            for idx, i in enumerate(iters):
                last_image = idx == len(iters) - 1
                # Flat fp8 image tile: 3 cin chunks side by side; one DMA
                # per chunk (HWDGE descriptor-gen is ~630ns per DMA, so
                # fewer, larger DMAs win), chunk 1 on the gpsimd SWDGE.
                xf = xin.tile([128, XFW], fp8, tag="x")
                pstride = list(xf.ap)[0][0]
                # One DMA per chunk on the SP queue. Splitting chunks does
                # not help the cold start: dependency tracking is interval-
                # based, so the cross-chunk pair-0 read range spans whole
                # chunks regardless.
                for ch in range(NCHUNK):
                    nc.sync.dma_start(
                        out=xf[:, ch * NPIX:(ch + 1) * NPIX],
                        in_=x[i, ch])
                if idx == 0:
                    load_small_consts()
                # Fill the ones region (pair 13's slot-B data, the conv-bias
                # fold). Rewritten per image so the tile framework tracks the
                # read dependency; emitted after the chunk DMAs so the
                # serial DMA pipe delivers the conv-critical data first.
                nc.sync.dma_start(out=xf[:, NCHUNK * NPIX:XFW], in_=ones[:])

                def conv(ptile, t, k, start, stop):
                    # 4-dim moving AP: [partition, k-pair, row, col] skips
                    # the 2 pad columns between rows, so only the 4096 valid
                    # output pixels are streamed (8 rows x 64 cols per tile).
                    s0 = START + t * ROWPITCH
                    delta = DELTA[k] if DELTA[k] is not None \
                        else NCHUNK * NPIX - (BASEA[k] + s0)
                    rhs = AP(xf.tensor, xf.offset + BASEA[k] + s0,
                             [[pstride, 128], [delta, 2],
                              [WP, ROWS_PT], [1, W]])
                    nc.tensor.matmul(
                        out=ptile[:, :TS],
                        lhsT=w1_s[:, k],
                        rhs=rhs,
                        start=start,
                        stop=stop,
                        perf_mode=DR,
                    )

                def relu(ptile, t, hh, g):
                    # h = max(p/4096, 0) = relu(p/4096 + b1) on the
                    # otherwise-idle DVE (b1 was folded into conv pair 13),
                    # written as fp8 for the DoubleRow 1x1.
                    n = TILE_N[t]
                    nc.vector.scalar_tensor_tensor(
                        out=hh[:, g * TS:g * TS + n], in0=ptile[:, :n],
                        scalar=DESCALE, in1=z_s[:, :n],
                        op0=mybir.AluOpType.mult, op1=mybir.AluOpType.max,
                    )
                    return (g, n, t)

                for gi, grp in enumerate(GROUPS):
                    ptiles = [ps1.tile([CHID, TS], f32, tag=f"p{g % 2}",
                                       name=f"p1_{g % 2}")
                              for g in grp]
                    # k-outer / tile-inner: one weight load per pair serves
                    # the whole group. (Palindromic pair order across groups
                    # was tried and reverted: it leads groups with pair 13,
                    # whose ones-region dependency lands last in each
                    # image's DMA sequence, costing 0.5us.)
                    for j, k in enumerate(range(NPAIR)):
                        for g, t in enumerate(grp):
                            conv(ptiles[g], t, k, j == 0, j == NPAIR - 1)
                    if pend is not None:
                        flush(pend)
                    hh = hbuf.tile([CHID, 2 * TS], fp8, tag="hh",
                                   name="hh")
                    spans = [relu(ptiles[g], t, hh, g)
                             for g, t in enumerate(grp)]
                    pend = (hh, spans, i)
            flush(pend)
    nc.compile()
    return nc


def _prep_inputs(dino_features, W1, b1, W2, b2):
    # Accept jax arrays / non-contiguous inputs as-is.
    dino_features = np.asarray(dino_features, dtype=np.float32)
    W1 = np.asarray(W1, dtype=np.float32)
    b1 = np.asarray(b1, dtype=np.float32)
    W2 = np.asarray(W2, dtype=np.float32)
    b2 = np.asarray(b2, dtype=np.float32)
    xp = np.zeros((B, HP, WP, CIN), dtype=np.float32)
    xp[:, 1:H + 1, 1:W + 1, :] = dino_features * SX
    # -> [B, chunk, cin_in_chunk, padded_pixel]
    xq = np.ascontiguousarray(
        xp.transpose(0, 3, 1, 2).reshape(B, NCHUNK, 128, NPIX)).astype(E4M3)

    # W1 (3,3,384,128) (ky,kx,ci,co) -> [chunk, cin128, tap, cout], then pair
    wq = (np.asarray(W1) * SW).astype(E4M3)
    wr = wq.transpose(2, 0, 1, 3).reshape(NCHUNK, 128, 9, CHID)
    w1p = np.zeros((128, NPAIR, 2, CHID), dtype=E4M3)
    for k, (cA, tA, cB, tB) in enumerate(PAIRS):
        w1p[:, k, 0, :] = wr[cA, :, tA, :]
        if cB is not None:
            w1p[:, k, 1, :] = wr[cB, :, tB, :]
        else:
            # Bias fold: slot B multiplies the all-ones region; each of the
            # 128 rows contributes (4096*b1)/128 = 32*b1 to PSUM.
            w1p[:, k, 1, :] = np.broadcast_to(
                (32.0 * np.asarray(b1).reshape(1, CHID)).astype(E4M3),
                (128, CHID))

    w2p = np.zeros((128, 2, 16), dtype=E4M3)
    w2p[:, 0, 0] = (np.asarray(W2).reshape(CHID) * SW2).astype(E4M3)
    b2h = np.full((97, 1), np.float32(b2.reshape(())), dtype=np.float32)
    onesh = np.ones((128, ONES_W), dtype=E4M3)

    in_maps = []
    for c in range(NCORES):
        in_maps.append({
            "x": np.ascontiguousarray(xq[c * BLOC:(c + 1) * BLOC]),
            "w1": w1p, "ones": onesh, "w2": w2p, "b2": b2h,
        })
    return in_maps


def kernel(dino_features, W1, b1, W2, b2, _trace=False, _trace_kwargs=None):
    if "nc" not in _CACHED:
        _CACHED["nc"] = _build_bass()
    nc = _CACHED["nc"]
    in_maps = _prep_inputs(dino_features, W1, b1, W2, b2)
    res = run_bass_kernel_spmd(nc, in_maps, core_ids=list(range(NCORES)),
                               trace=_trace, **(_trace_kwargs or {}))
    _CACHED["last_results"] = res
    out = np.concatenate([res.results[c]["y"] for c in range(NCORES)], axis=0)
    # y is densely packed: flat col 64*h + w -> pixel (h, w).
    return np.ascontiguousarray(out).reshape(B, H, W, 1).astype(np.float32)



# revision 6
# speedup vs baseline: 1.0638x; 1.0638x over previous
"""Trainium2 Bass kernel for KeypointSelector:
conv3x3(384->128, pad 1) + bias + ReLU -> conv1x1(128->1) + bias + sigmoid.

Input  dino_features: (32, 64, 64, 384) f32
Output (32, 64, 64, 1) f32

Strategy: pure data parallel over batch, 4 images per core on 8 cores.
The 3x3 conv runs in fp8e4m3 with MatmulPerfMode.DoubleRow (0.5 PE
cycles per output column): the 27 contraction k-tiles (3 cin chunks x 9
taps) pack into 14 DoubleRow pairs (the 28th slot carries zero weights).

Block-interleaved input layout: each image is stored as 8 row-blocks of
[128, 1980] fp8 -- padded rows [8t, 8t+10) of chunk0|chunk1|chunk2 side
by side (660 cols each). Every conv output tile t (8 rows x 64 cols =
512 px) reads ONLY block t, so one small DMA unblocks the first matmuls
~3us earlier than whole-chunk loads, and all DoubleRow pair deltas stay
in [596, 1387] > the 526-col moving window (no overlapping reads).

The pointwise tail runs entirely off the PE:
- DVE: g = max(psum, -SX*SW*b1) * (W2*DESCALE) per channel, one
  scalar_tensor_tensor (per-partition scalar AP + stride-0 broadcast).
  Identity: sum_c W2_c*(max(conv_c, -b1_c) + b1_c) = conv1x1(relu(...)),
  with the constant sum_c W2_c*b1_c folded into the sigmoid bias.
- GpSimd: partition_all_reduce(add) sums the 128 channels.
- ACT: sigmoid with bias b2 + sum(W2*b1).
This removes the old PE 1x1 matmuls (~3.4us of PE time); the PE now runs
only the conv (14 pairs x 8 tiles x 4 images = 114688 cycles ~ 47.8us).

Cold start: w1 is split in two DMAs around image0/block0 so the first
matmul needs only 1.3us of serial DMA; image 0 uses single-tile groups
so tile 1's data cannot stall the k-loop; the PE p-state ramp is primed
with matmuls on a DVE-memset zeros tile (DVE, not ACT: the ACT-engine
memzero hides a 1.3us LoadActFuncSet on the critical path).
Tail: the last image ends with a 2-row (128 px) segment so the final
relu/reduce/sigmoid/DMA chain is short.
"""

import ml_dtypes
import numpy as np

import concourse.tile as tile
from concourse import bacc, bass_isa, mybir
from concourse.ap import AP
from concourse.bass_utils import run_bass_kernel_spmd

E4M3 = ml_dtypes.float8_e4m3  # == mybir.dt.np(float8e4)

# Geometry
B, H, W, CIN, CHID = 32, 64, 64, 384, 128
NCORES = 8
BLOC = B // NCORES  # 4 images per core
BW = W + 2  # padded row width (66)
NCHUNK = CIN // 128  # 3 cin chunks
TS = 512  # matmul free-dim tile (one PSUM bank of fp32)
ROWS_PT = 8  # output rows per tile
NT = H // ROWS_PT  # 8 tiles per image
BROWS = ROWS_PT + 2  # padded rows per block (halo)
CHW = BROWS * BW  # chunk width inside a block (660)
BLKW = NCHUNK * CHW  # block width (1980)
XFW = NT * BLKW  # flat image tile width (15840)
OUTW = H * W  # densely packed flat output row (4096)

# fp8 scaling
SX, SW = 16.0, 256.0
DESCALE = 1.0 / (SX * SW)

# In-block offset of tap t=(dy+1)*3+(dx+1): (1+dy)*BW + (1+dx).
TOFF = [0, 1, 2, BW, BW + 1, BW + 2, 2 * BW, 2 * BW + 1, 2 * BW + 2]

# k-tile pairing for DoubleRow: 27 (chunk, tap) k-tiles -> 13 full pairs
# + 1 half pair whose slot A reads (c0, t1) with zero weights (the data
# is finite fp8, so 0*x contributes 0 to PSUM). Two hardware constraints
# on the slot A -> slot B rhs delta, found empirically (the device takes
# NRT_EXEC_UNIT_UNRECOVERABLE otherwise):
#   1. delta > 526 (the 8x64 moving window span) -- no overlapping reads;
#   2. delta must be EVEN: odd deltas crash the DoubleRow dual-stream
#      ifmap fetch. TOFF parity is odd exactly for taps t%3==1 (dx=0),
#      so pairs match taps of equal dx-parity across chunks.
PAIRS = [
    (0, 0, 1, 0), (0, 2, 1, 2), (0, 3, 1, 3), (0, 1, 1, 1), (0, 4, 1, 4),
    (0, 5, 2, 0), (0, 6, 2, 2), (0, 8, 2, 3), (0, 7, 2, 1),
    (1, 5, 2, 5), (1, 6, 2, 6), (1, 8, 2, 8), (1, 7, 2, 4),
    (None, 1, 2, 7),  # slot A zero-weighted
]
NPAIR = len(PAIRS)  # 14
BASEA, DELTA = [], []
for cA, tA, cB, tB in PAIRS:
    ca = 0 if cA is None else cA
    BASEA.append(ca * CHW + TOFF[tA])
    DELTA.append(cB * CHW + TOFF[tB] - BASEA[-1])
assert all(d > 7 * BW + W and d % 2 == 0 for d in DELTA), DELTA
assert all(BASEA[k] + DELTA[k] + 7 * BW + W - 1 < BLKW for k in range(NPAIR))
_taps = sorted((c, t) for cA, tA, cB, tB in PAIRS
               for c, t in ([(cA, tA)] if cA is not None else []) + [(cB, tB)])
assert _taps == sorted((c, t) for c in range(3) for t in range(9)), _taps

W1SPLIT = 7  # w1 pairs [0, 7) land before block 0, the rest after

_CACHED = {}


def _build_bass(reps=1):
    nc = bacc.Bacc("TRN2", target_bir_lowering=False)

    f32 = mybir.dt.float32
    bf16 = mybir.dt.bfloat16
    fp8 = mybir.dt.float8e4
    DR = mybir.MatmulPerfMode.DoubleRow

    x = nc.dram_tensor("x", [BLOC, NT, 128, BLKW], fp8, kind="ExternalInput")
    w1 = nc.dram_tensor("w1", [128, NPAIR, 2, CHID], fp8, kind="ExternalInput")
    w2 = nc.dram_tensor("w2", [128, 1], f32, kind="ExternalInput")
    nb = nc.dram_tensor("nb", [128, 1], f32, kind="ExternalInput")
    b2 = nc.dram_tensor("b2", [97, 1], f32, kind="ExternalInput")
    y = nc.dram_tensor("y", [BLOC, OUTW], f32, kind="ExternalOutput")

    with tile.TileContext(nc) as tc:
        with (
            tc.tile_pool(name="consts", bufs=1) as consts,
            tc.tile_pool(name="xin", bufs=2) as xin,
            tc.tile_pool(name="gbuf", bufs=2) as gbuf,
            tc.tile_pool(name="rbuf", bufs=2) as rbuf,
            tc.tile_pool(name="obuf", bufs=2) as obuf,
            tc.tile_pool(name="ps1", bufs=2, space="PSUM") as ps1,
            tc.tile_pool(name="psp", bufs=1, space="PSUM") as psp,
        ):
            # w1 first half on the SP queue ahead of image 0 block 0; the
            # first Ldweights needs it. Second half lands after block 0.
            w1_s = consts.tile([128, NPAIR, 2, CHID], fp8)
            nc.sync.dma_start(out=w1_s[:, 0:W1SPLIT], in_=w1[:, 0:W1SPLIT])
            # Tiny pointwise consts ride the ACT queue (7ns copies, their
            # HWDGE gens interleave without displacing block DMAs).
            w2_s = consts.tile([128, 1], f32)
            nb_s = consts.tile([128, 1], f32)
            b2_s = consts.tile([97, 1], f32)
            nc.scalar.dma_start(out=w2_s, in_=w2[:])
            nc.scalar.dma_start(out=nb_s, in_=nb[:])
            nc.scalar.dma_start(out=b2_s, in_=b2[:])
            # Zeros tile for the p-state priming matmuls, zeroed on the
            # otherwise-idle DVE (ACT memzero would queue behind a 1.3us
            # LoadActFuncSet; Pool runs the framework preamble).
            z_s = consts.tile([CHID, TS], bf16)
            nc.vector.memset(z_s[:], 0.0)

            # Prime the PE p-state ramp (full clock needs ~3us since
            # pe_busy_start) with throwaway matmuls while DMAs land:
            # 6 x 512-col to span the ramp, then 128-col filler so the
            # queue drains quickly once real data is ready.
            prime = psp.tile([CHID, TS], f32, name="prime", tag="pp")
            for _ in range(6):
                nc.tensor.matmul(out=prime[:, :TS], lhsT=z_s[:, 0:CHID],
                                 rhs=z_s[:, 0:TS], start=True, stop=True)
            for _ in range(4):
                nc.tensor.matmul(out=prime[:, :128], lhsT=z_s[:, 0:CHID],
                                 rhs=z_s[:, 0:128], start=True, stop=True)

            # Segment = (tile, row_off, nrows): conv output rows
            # [8*tile+row_off, +nrows) x 64 cols, n = nrows*64 columns.
            def segs_for(first, last):
                if last:
                    segs = [(t, 0, 8) for t in range(NT - 1)]
                    segs += [(7, 0, 6), (7, 6, 2)]
                else:
                    segs = [(t, 0, 8) for t in range(NT)]
                if first:
                    # Single-seg leading groups: tile 1's block cannot
                    # stall tile 0's k-loop during the cold start.
                    groups = [[segs[0]], [segs[1]]] + [
                        segs[i:i + 2] for i in range(2, len(segs), 2)]
                elif last:
                    # Tiny trailing group shortens the flush tail.
                    groups = [segs[i:i + 2] for i in range(0, 6, 2)]
                    groups += [[segs[6], segs[7]], [segs[8]]]
                else:
                    groups = [segs[i:i + 2] for i in range(0, NT, 2)]
                return groups

            # One-group software pipeline: group g's pointwise chain is
            # emitted after group g+1's conv matmuls so the PE never
            # queues behind the DVE/Pool/ACT engines.
            pend = None

            def flush(pend):
                gg, rr, spans, ptiles, img = pend
                w2b = AP(w2_s.tensor, w2_s.offset,
                         [list(w2_s.ap)[0], [0, TS]])
                for g, n, ypos in spans:
                    # g = max(psum, -SX*SW*b1[c]) * (W2[c]/4096) on DVE:
                    # per-partition scalar AP for the bias, stride-0
                    # broadcast of the 1x1 weight along the free dim.
                    nc.vector.scalar_tensor_tensor(
                        out=gg[:, g * TS:g * TS + n],
                        in0=ptiles[g][:, :n],
                        scalar=nb_s[:, 0:1],
                        in1=AP(w2b.tensor, w2b.offset,
                               [list(w2_s.ap)[0], [0, n]]),
                        op0=mybir.AluOpType.max,
                        op1=mybir.AluOpType.mult,
                    )
                for g, n, ypos in spans:
                    # Channel sum on the otherwise-idle GpSimd engine.
                    nc.gpsimd.partition_all_reduce(
                        out_ap=rr[:, g * TS:g * TS + n],
                        in_ap=gg[:, g * TS:g * TS + n],
                        channels=128,
                        reduce_op=bass_isa.ReduceOp.add,
                    )
                out_g = obuf.tile([97, TS], f32, name="og", tag="og")
                ostr = list(out_g.ap)[0][0]
                for g, n, ypos in spans:
                    nc.scalar.activation(
                        out=out_g[32 * g:32 * g + 1, :n],
                        in_=rr[32 * g:32 * g + 1, g * TS:g * TS + n],
                        func=mybir.ActivationFunctionType.Sigmoid,
                        bias=b2_s[32 * g:32 * g + 1], scale=1.0,
                    )
                full = [(g, ypos) for g, n, ypos in spans if n == TS]
                part = [(g, n, ypos) for g, n, ypos in spans if n != TS]
                if full:
                    # DMA APs may stride partitions (engine APs may not):
                    # one strided DMA writes the group's full tiles.
                    m = len(full)
                    y0 = full[0][1]
                    nc.sync.dma_start(
                        out=y[img, y0:y0 + m * TS],
                        in_=AP(out_g.tensor, out_g.offset,
                               [[32 * ostr, m], [1, TS]]))
                for g, n, ypos in part:
                    nc.sync.dma_start(
                        out=y[img, ypos:ypos + n],
                        in_=out_g[32 * g:32 * g + 1, :n])

            iters = [ii for _ in range(reps) for ii in range(BLOC)]
            for idx, i in enumerate(iters):
                first = idx == 0
                last = idx == len(iters) - 1
                xf = xin.tile([128, XFW], fp8, tag="x")
                pstride = list(xf.ap)[0][0]
                # One DMA per block; block t alone feeds output tile t.
                for t in range(NT):
                    nc.sync.dma_start(
                        out=xf[:, t * BLKW:(t + 1) * BLKW],
                        in_=x[i, t])
                    if first and t == 0:
                        nc.sync.dma_start(out=w1_s[:, W1SPLIT:NPAIR],
                                          in_=w1[:, W1SPLIT:NPAIR])

                def conv(ptile, seg, k, start, stop):
                    t, row_off, nrows = seg
                    base = t * BLKW + BASEA[k] + row_off * BW
                    rhs = AP(xf.tensor, xf.offset + base,
                             [[pstride, 128], [DELTA[k], 2],
                              [BW, nrows], [1, W]])
                    nc.tensor.matmul(
                        out=ptile[:, :nrows * W],
                        lhsT=w1_s[:, k],
                        rhs=rhs,
                        start=start,
                        stop=stop,
                        perf_mode=DR,
                    )

                for grp in segs_for(first, last):
                    ptiles = [ps1.tile([CHID, TS], f32, tag=f"p{g % 2}",
                                       name=f"p1_{g % 2}")
                              for g in range(len(grp))]
                    # k-outer / seg-inner: one weight load per pair
                    # serves the whole group.
                    for k in range(NPAIR):
                        for g, seg in enumerate(grp):
                            conv(ptiles[g], seg, k, k == 0, k == NPAIR - 1)
                    if pend is not None:
                        flush(pend)
                    gg = gbuf.tile([CHID, 2 * TS], f32, tag="gg", name="gg")
                    rr = rbuf.tile([128, 2 * TS], f32, tag="rr", name="rr")
                    spans = [(g, nrows * W, t * TS + row_off * W)
                             for g, (t, row_off, nrows) in enumerate(grp)]
                    pend = (gg, rr, spans, ptiles, i)
            flush(pend)
    nc.compile()
    return nc


def _prep_inputs(dino_features, W1, b1, W2, b2):
    dino_features = np.asarray(dino_features, dtype=np.float32)
    W1 = np.asarray(W1, dtype=np.float32)
    b1 = np.asarray(b1, dtype=np.float32).reshape(CHID)
    W2 = np.asarray(W2, dtype=np.float32).reshape(CHID)
    b2 = np.float32(np.asarray(b2, dtype=np.float32).reshape(()))

    # Pad, scale, cast to fp8 once, then gather overlapping row-blocks.
    xp = np.zeros((B, BW, BW, CIN), dtype=np.float32)
    xp[:, 1:H + 1, 1:W + 1, :] = dino_features * SX
    xt = np.ascontiguousarray(xp.transpose(0, 3, 1, 2)).astype(E4M3)
    xt = xt.reshape(B, NCHUNK, 128, BW, BW)
    xb = np.empty((B, NT, 128, NCHUNK, CHW), dtype=E4M3)
    for t in range(NT):
        blk = xt[:, :, :, 8 * t:8 * t + BROWS, :].reshape(
            B, NCHUNK, 128, CHW)
        xb[:, t] = blk.transpose(0, 2, 1, 3)
    xb = xb.reshape(B, NT, 128, BLKW)

    # W1 (3,3,384,128) (ky,kx,ci,co) -> [chunk, cin128, tap, cout] pairs.
    wq = (W1 * SW).astype(E4M3)
    wr = wq.transpose(2, 0, 1, 3).reshape(NCHUNK, 128, 9, CHID)
    w1p = np.zeros((128, NPAIR, 2, CHID), dtype=E4M3)
    for k, (cA, tA, cB, tB) in enumerate(PAIRS):
        if cA is not None:
            w1p[:, k, 0, :] = wr[cA, :, tA, :]
        w1p[:, k, 1, :] = wr[cB, :, tB, :]

    w2h = (W2 * DESCALE).astype(np.float32).reshape(128, 1)
    nbh = (-(SX * SW) * b1).astype(np.float32).reshape(128, 1)
    b2h = np.full((97, 1), b2 + np.float32(np.dot(W2, b1)), dtype=np.float32)

    in_maps = []
    for c in range(NCORES):
        in_maps.append({
            "x": np.ascontiguousarray(xb[c * BLOC:(c + 1) * BLOC]),
            "w1": w1p, "w2": w2h, "nb": nbh, "b2": b2h,
        })
    return in_maps


def kernel(dino_features, W1, b1, W2, b2, _trace=False, _trace_kwargs=None):
    if "nc" not in _CACHED:
        _CACHED["nc"] = _build_bass()
    nc = _CACHED["nc"]
    in_maps = _prep_inputs(dino_features, W1, b1, W2, b2)
    res = run_bass_kernel_spmd(nc, in_maps, core_ids=list(range(NCORES)),
                               trace=_trace, **(_trace_kwargs or {}))
    _CACHED["last_results"] = res
    out = np.concatenate([res.results[c]["y"] for c in range(NCORES)], axis=0)
    # y is densely packed: flat col 64*h + w -> pixel (h, w).
    return np.ascontiguousarray(out).reshape(B, H, W, 1).astype(np.float32)


# revision 33
# speedup vs baseline: 1.1159x; 1.0490x over previous
"""Trainium2 Bass kernel for KeypointSelector:
conv3x3(384->128, pad 1) + bias + ReLU -> conv1x1(128->1) + bias + sigmoid.

Input  dino_features: (32, 64, 64, 384) f32
Output (32, 64, 64, 1) f32

Strategy: pure data parallel over batch, 4 images per core on 8 cores.
The 3x3 conv runs in fp8e4m3 with MatmulPerfMode.DoubleRow (0.5 PE
cycles per output column): the 27 contraction k-tiles (3 cin chunks x 9
taps) pack into 14 DoubleRow pairs (the 28th slot carries zero weights).

Block-interleaved input layout: each image is stored as 8 row-blocks of
[128, 1980] fp8 -- padded rows [8t, 8t+10) of chunk0|chunk1|chunk2 side
by side (660 cols each). Every conv output tile t (8 rows x 64 cols =
512 px) reads ONLY block t, so one small DMA unblocks the first matmuls
~3us earlier than whole-chunk loads, and all DoubleRow pair deltas stay
in [596, 1387] > the 526-col moving window (no overlapping reads).

The pointwise tail runs entirely off the PE:
- DVE: g = max(psum, -SX*SW*b1) * (W2*DESCALE) per channel, one
  scalar_tensor_tensor (per-partition scalar AP + stride-0 broadcast).
  Identity: sum_c W2_c*(max(conv_c, -b1_c) + b1_c) = conv1x1(relu(...)),
  with the constant sum_c W2_c*b1_c folded into the sigmoid bias.
- GpSimd: partition_all_reduce(add) sums the 128 channels.
- ACT: sigmoid with bias b2 + sum(W2*b1).
This removes the old PE 1x1 matmuls (~3.4us of PE time); the PE now runs
only the conv (14 pairs x 8 tiles x 4 images = 114688 cycles ~ 47.8us).

Cold start: w1 is split in two DMAs around image0/block0 so the first
matmul needs only 1.3us of serial DMA; image 0 uses single-tile groups
so tile 1's data cannot stall the k-loop; the PE p-state ramp is primed
with matmuls on a DVE-memset zeros tile (DVE, not ACT: the ACT-engine
memzero hides a 1.3us LoadActFuncSet on the critical path).
Tail: the last image ends with a 2-row (128 px) segment so the final
relu/reduce/sigmoid/DMA chain is short.
"""

import ml_dtypes
import numpy as np

import concourse.tile as tile
from concourse import bacc, bass_isa, mybir
from concourse.ap import AP
from concourse.bass_utils import run_bass_kernel_spmd

E4M3 = ml_dtypes.float8_e4m3  # == mybir.dt.np(float8e4)

# Geometry
B, H, W, CIN, CHID = 32, 64, 64, 384, 128
NCORES = 8
BLOC = B // NCORES  # 4 images per core
BW = W + 2  # padded row width (66)
NCHUNK = CIN // 128  # 3 cin chunks
TS = 512  # matmul free-dim tile (one PSUM bank of fp32)
ROWS_PT = 8  # output rows per tile
NT = H // ROWS_PT  # 8 tiles per image
BROWS = ROWS_PT + 2  # padded rows per block (halo)
CHW = BROWS * BW  # chunk width inside a block (660)
BLKW = NCHUNK * CHW  # block width (1980)
XFW = NT * BLKW  # flat image tile width (15840)
OUTW = H * W  # densely packed flat output row (4096)

# fp8 scaling
SX, SW = 16.0, 256.0
DESCALE = 1.0 / (SX * SW)

# In-block offset of tap t=(dy+1)*3+(dx+1): (1+dy)*BW + (1+dx).
TOFF = [0, 1, 2, BW, BW + 1, BW + 2, 2 * BW, 2 * BW + 1, 2 * BW + 2]

# k-tile pairing for DoubleRow: 27 (chunk, tap) k-tiles -> 13 full pairs
# + 1 half pair whose slot A reads (c0, t1) with zero weights (the data
# is finite fp8, so 0*x contributes 0 to PSUM). Two hardware constraints
# on the slot A -> slot B rhs delta, found empirically (the device takes
# NRT_EXEC_UNIT_UNRECOVERABLE otherwise):
#   1. delta > 526 (the 8x64 moving window span) -- no overlapping reads;
#   2. delta must be EVEN: odd deltas crash the DoubleRow dual-stream
#      ifmap fetch. TOFF parity is odd exactly for taps t%3==1 (dx=0),
#      so pairs match taps of equal dx-parity across chunks.
PAIRS = [
    (0, 0, 1, 0), (0, 2, 1, 2), (0, 3, 1, 3), (0, 1, 1, 1), (0, 4, 1, 4),
    (0, 5, 2, 0), (0, 6, 2, 2), (0, 8, 2, 3), (0, 7, 2, 1),
    (1, 5, 2, 5), (1, 6, 2, 6), (1, 8, 2, 8), (1, 7, 2, 4),
    (None, 1, 2, 7),  # slot A zero-weighted
]
NPAIR = len(PAIRS)  # 14
BASEA, DELTA = [], []
for cA, tA, cB, tB in PAIRS:
    ca = 0 if cA is None else cA
    BASEA.append(ca * CHW + TOFF[tA])
    DELTA.append(cB * CHW + TOFF[tB] - BASEA[-1])
assert all(d > 7 * BW + W and d % 2 == 0 for d in DELTA), DELTA
assert all(BASEA[k] + DELTA[k] + 7 * BW + W - 1 < BLKW for k in range(NPAIR))
_taps = sorted((c, t) for cA, tA, cB, tB in PAIRS
               for c, t in ([(cA, tA)] if cA is not None else []) + [(cB, tB)])
assert _taps == sorted((c, t) for c in range(3) for t in range(9)), _taps

W1SPLIT = 7  # w1 pairs [0, 7) land before block 0, the rest after

_CACHED = {}


def _build_bass(reps=1):
    nc = bacc.Bacc("TRN2", target_bir_lowering=False)

    f32 = mybir.dt.float32
    bf16 = mybir.dt.bfloat16
    fp8 = mybir.dt.float8e4
    DR = mybir.MatmulPerfMode.DoubleRow

    x = nc.dram_tensor("x", [BLOC, NT, 128, BLKW], fp8, kind="ExternalInput")
    w1 = nc.dram_tensor("w1", [128, NPAIR, 2, CHID], fp8, kind="ExternalInput")
    cst = nc.dram_tensor("cst", [128, 3], f32, kind="ExternalInput")
    y = nc.dram_tensor("y", [BLOC, OUTW], f32, kind="ExternalOutput")

    with tile.TileContext(nc) as tc:
        with (
            tc.tile_pool(name="consts", bufs=1) as consts,
            tc.tile_pool(name="xin", bufs=2) as xin,
            tc.tile_pool(name="gbuf", bufs=4) as gbuf,
            tc.tile_pool(name="rbuf", bufs=4) as rbuf,
            # obuf depth 4: with 2 buffers the tail sigmoids serialize on
            # the y DMA + 900ns sem-prop of the group two flushes back.
            tc.tile_pool(name="obuf", bufs=4) as obuf,
            tc.tile_pool(name="ps1", bufs=3, space="PSUM") as ps1,
            tc.tile_pool(name="psp", bufs=1, space="PSUM") as psp,
        ):
            # w1 first half on the SP queue ahead of image 0 block 0; the
            # first Ldweights needs it. Second half lands after block 0.
            w1_s = consts.tile([128, NPAIR, 2, CHID], fp8)
            nc.sync.dma_start(out=w1_s[:, 0:W1SPLIT], in_=w1[:, 0:W1SPLIT])
            # Pointwise consts (w2*DESCALE | -SX*SW*b1 | sigmoid bias) as ONE
            # tiny DMA, emitted on the sync queue after w1b: each HWDGE
            # descriptor-gen costs 625ns of the serial gen pipe, and a
            # second-queue DMA would slot its gen between w1a and block 0,
            # delaying the first conv matmul by that much.
            cst_s = consts.tile([128, 3], f32)
            w2_s = cst_s[:, 0:1]
            nb_s = cst_s[:, 1:2]
            # Zeros tile for the p-state priming matmuls, zeroed on the
            # otherwise-idle DVE (ACT memzero would queue behind a 1.3us
            # LoadActFuncSet; Pool runs the framework preamble).
            z_s = consts.tile([CHID, 256], bf16)
            nc.vector.memset(z_s[:], 0.0)

            # Prime the PE p-state ramp (full clock needs ~3us since
            # pe_busy_start) with throwaway matmuls while DMAs land,
            # sized to end right as block 0 becomes readable (~4.2us).
            prime = psp.tile([CHID, TS], f32, name="prime", tag="pp")
            for _ in range(12):
                nc.tensor.matmul(out=prime[:, :256], lhsT=z_s[:, 0:CHID],
                                 rhs=z_s[:, 0:256], start=True, stop=True)
            for _ in range(2):
                nc.tensor.matmul(out=prime[:, :64], lhsT=z_s[:, 0:CHID],
                                 rhs=z_s[:, 0:64], start=True, stop=True)

            # Segment = (tile, row_off, nrows): conv output rows
            # [8*tile+row_off, +nrows) x 64 cols, n = nrows*64 columns.
            def segs_for(first, last):
                if first:
                    # Single-seg leading groups: tile 1's block cannot
                    # stall tile 0's k-loop during the cold start.
                    return [[(0, 0, 8)], [(1, 0, 8)]] + [
                        [(t, 0, 8), (t + 1, 0, 8)] for t in (2, 4, 6)]
                if last:
                    # Four equal 256-px trailing groups: each flush can
                    # only start when its whole k-loop ends, so the tail
                    # chains get conv cover from the groups after them.
                    # They share one out_g (rows 0/32/64/96) so a SINGLE
                    # strided y DMA runs after the last sigmoid -- tail
                    # descriptor-gens don't stack on the serial HWDGE.
                    return ([[(t, 0, 8), (t + 1, 0, 8)] for t in (0, 2, 4)]
                            + [[(6, 0, 4)], [(6, 4, 4)], [(7, 0, 4)],
                               [(7, 4, 4)]])
                return [[(t, 0, 8), (t + 1, 0, 8)] for t in (0, 2, 4, 6)]

            # Each group's pointwise chain is emitted right after its conv
            # matmuls: the chain contains no PE instructions (DVE -> GpSimd
            # -> ACT -> DMA), so it runs under the NEXT group's conv window
            # and only the final tiny group's chain trails the last matmul.
            def flush(pend, out_g=None, grow=0, emit=True):
                gg, rr, spans, ptiles, img = pend
                pdim = list(cst_s.ap)[0]
                for g, n, ypos in spans:
                    # g = max(psum, -SX*SW*b1[c]) * (W2[c]/4096) on DVE:
                    # per-partition scalar AP for the bias, stride-0
                    # broadcast of the 1x1 weight along the free dim.
                    nc.vector.scalar_tensor_tensor(
                        out=gg[:, g * TS:g * TS + n],
                        in0=ptiles[g][:, :n],
                        scalar=nb_s,
                        in1=AP(w2_s.tensor, w2_s.offset, [pdim, [0, n]]),
                        op0=mybir.AluOpType.max,
                        op1=mybir.AluOpType.mult,
                    )
                for g, n, ypos in spans:
                    # Channel sum on the otherwise-idle GpSimd engine.
                    nc.gpsimd.partition_all_reduce(
                        out_ap=rr[:, g * TS:g * TS + n],
                        in_ap=gg[:, g * TS:g * TS + n],
                        channels=128,
                        reduce_op=bass_isa.ReduceOp.add,
                    )
                if out_g is None:
                    out_g = obuf.tile([97, TS], f32, name="og", tag="og")
                ostr = list(out_g.ap)[0][0]
                rows = []
                for g, n, ypos in spans:
                    r = 32 * (grow + g)
                    rows.append((r, n, ypos))
                    nc.scalar.activation(
                        out=out_g[r:r + 1, :n],
                        in_=rr[r:r + 1, g * TS:g * TS + n],
                        func=mybir.ActivationFunctionType.Sigmoid,
                        bias=cst_s[r:r + 1, 2:3], scale=1.0,
                    )
                if not emit:
                    return
                # DMA APs may stride partitions (engine APs may not): one
                # strided DMA per run of equal-length rows.
                i0 = 0
                while i0 < len(rows):
                    i1 = i0 + 1
                    while (i1 < len(rows) and rows[i1][1] == rows[i0][1]
                           and rows[i1][0] == rows[i1 - 1][0] + 32):
                        i1 += 1
                    m = i1 - i0
                    r0, n, y0 = rows[i0]
                    nc.sync.dma_start(
                        out=y[img, y0:y0 + m * n],
                        in_=AP(out_g.tensor, out_g.offset + r0 * ostr,
                               [[32 * ostr, m], [1, n]]))
                    i0 = i1

            iters = [ii for _ in range(reps) for ii in range(BLOC)]
            for idx, i in enumerate(iters):
                first = idx == 0
                last = idx == len(iters) - 1
                xf = xin.tile([128, XFW], fp8, tag="x")
                pstride = list(xf.ap)[0][0]
                # Block 0 splits at the chunk-2 boundary (tile 0's first
                # conv pairs read only chunks 0-1, so the first matmul can
                # start 235ns earlier); block 1 ships alone; later blocks
                # ship in pairs -- each HWDGE gen costs 625ns of the serial
                # gen pipe, and DMA bandwidth leads the PE by 2x, so fewer
                # gens beat finer intervals.
                for t0, nb_ in ((0, 1), (1, 1), (2, 2), (4, 2), (6, 2)):
                    xi = x[i, t0]
                    nc.sync.dma_start(
                        out=xf[:, t0 * BLKW:(t0 + nb_) * BLKW],
                        in_=AP(xi.tensor, xi.offset,
                               [[BLKW, 128], [128 * BLKW, nb_], [1, BLKW]]))
                    if first and t0 == 0:
                        nc.sync.dma_start(out=w1_s[:, W1SPLIT:NPAIR],
                                          in_=w1[:, W1SPLIT:NPAIR])
                        nc.sync.dma_start(out=cst_s, in_=cst[:])

                def conv(ptile, seg, k, start, stop):
                    t, row_off, nrows = seg
                    base = t * BLKW + BASEA[k] + row_off * BW
                    rhs = AP(xf.tensor, xf.offset + base,
                             [[pstride, 128], [DELTA[k], 2],
                              [BW, nrows], [1, W]])
                    nc.tensor.matmul(
                        out=ptile[:, :nrows * W],
                        lhsT=w1_s[:, k],
                        rhs=rhs,
                        start=start,
                        stop=stop,
                        perf_mode=DR,
                    )

                groups = segs_for(first, last)
                ntail = 4 if last else 0
                out_tail = None
                for gi, grp in enumerate(groups):
                    ptiles = [ps1.tile([CHID, TS], f32, tag=f"p{g % 2}",
                                       name=f"p1_{g % 2}")
                              for g in range(len(grp))]
                    # k-outer / seg-inner: one weight load per pair
                    # serves the whole group.
                    for k in range(NPAIR):
                        for g, seg in enumerate(grp):
                            conv(ptiles[g], seg, k, k == 0, k == NPAIR - 1)
                    gg = gbuf.tile([CHID, 2 * TS], f32, tag="gg", name="gg")
                    rr = rbuf.tile([128, 2 * TS], f32, tag="rr", name="rr")
                    spans = [(g, nrows * W, t * TS + row_off * W)
                             for g, (t, row_off, nrows) in enumerate(grp)]
                    ti = gi - (len(groups) - ntail)
                    if ti < 0:
                        flush((gg, rr, spans, ptiles, i))
                        continue
                    if out_tail is None:
                        out_tail = obuf.tile([97, TS], f32, name="og",
                                             tag="og")
                    flush((gg, rr, spans, ptiles, i), out_g=out_tail,
                          grow=ti, emit=False)
                    if ti == ntail - 1:
                        # One strided DMA covers all four tail rows.
                        ostr = list(out_tail.ap)[0][0]
                        y0 = 4096 - ntail * 256
                        nc.sync.dma_start(
                            out=y[i, y0:4096],
                            in_=AP(out_tail.tensor, out_tail.offset,
                                   [[32 * ostr, ntail], [1, 256]]))
    nc.compile()
    return nc


def _prep_inputs(dino_features, W1, b1, W2, b2):
    dino_features = np.asarray(dino_features, dtype=np.float32)
    W1 = np.asarray(W1, dtype=np.float32)
    b1 = np.asarray(b1, dtype=np.float32).reshape(CHID)
    W2 = np.asarray(W2, dtype=np.float32).reshape(CHID)
    b2 = np.float32(np.asarray(b2, dtype=np.float32).reshape(()))

    # Pad, scale, cast to fp8 once, then gather overlapping row-blocks.
    xp = np.zeros((B, BW, BW, CIN), dtype=np.float32)
    xp[:, 1:H + 1, 1:W + 1, :] = dino_features * SX
    xt = np.ascontiguousarray(xp.transpose(0, 3, 1, 2)).astype(E4M3)
    xt = xt.reshape(B, NCHUNK, 128, BW, BW)
    xb = np.empty((B, NT, 128, NCHUNK, CHW), dtype=E4M3)
    for t in range(NT):
        blk = xt[:, :, :, 8 * t:8 * t + BROWS, :].reshape(
            B, NCHUNK, 128, CHW)
        xb[:, t] = blk.transpose(0, 2, 1, 3)
    xb = xb.reshape(B, NT, 128, BLKW)

    # W1 (3,3,384,128) (ky,kx,ci,co) -> [chunk, cin128, tap, cout] pairs.
    wq = (W1 * SW).astype(E4M3)
    wr = wq.transpose(2, 0, 1, 3).reshape(NCHUNK, 128, 9, CHID)
    w1p = np.zeros((128, NPAIR, 2, CHID), dtype=E4M3)
    for k, (cA, tA, cB, tB) in enumerate(PAIRS):
        if cA is not None:
            w1p[:, k, 0, :] = wr[cA, :, tA, :]
        w1p[:, k, 1, :] = wr[cB, :, tB, :]

    csth = np.empty((128, 3), dtype=np.float32)
    csth[:, 0] = W2 * DESCALE
    csth[:, 1] = -(SX * SW) * b1
    csth[:, 2] = b2 + np.float32(np.dot(W2, b1))

    in_maps = []
    for c in range(NCORES):
        in_maps.append({
            "x": np.ascontiguousarray(xb[c * BLOC:(c + 1) * BLOC]),
            "w1": w1p, "cst": csth,
        })
    return in_maps


def kernel(dino_features, W1, b1, W2, b2, _trace=False, _trace_kwargs=None):
    if "nc" not in _CACHED:
        _CACHED["nc"] = _build_bass()
    nc = _CACHED["nc"]
    in_maps = _prep_inputs(dino_features, W1, b1, W2, b2)
    res = run_bass_kernel_spmd(nc, in_maps, core_ids=list(range(NCORES)),
                               trace=_trace, **(_trace_kwargs or {}))
    _CACHED["last_results"] = res
    out = np.concatenate([res.results[c]["y"] for c in range(NCORES)], axis=0)
    # y is densely packed: flat col 64*h + w -> pixel (h, w).
    return np.ascontiguousarray(out).reshape(B, H, W, 1).astype(np.float32)
